# revision 118
# baseline (speedup 1.0000x reference)
"""FAVOR+ (Performer) multi-head causal attention — Trainium2 Bass kernel v2.

Sharding: 8 cores = 4 batches x 2 head-groups (4 heads each).

Math note: the softmax-kernel stabilizers and +eps only rescale qp/kp per
(l,h) [or globally] and cancel in num/den (O(3e-4) perturbation). We compute
raw exp(dd) for Q and exp(dd - diag_k) for K; no collectives needed.

v2 design:
  * All scan-side math in bf16 (PSUM accumulates f32). Host ships x and
    weights pre-converted to bf16: no on-chip rounding copies, half the DMA.
    Validated numerically: 4.8e-3 rel err vs f32 reference (tol 2e-2).
  * All small constants packed into ONE blob DMA (SP SEQ issues DMAs at
    565ns each, so DMA count gates the pipeline start).
  * Hierarchical scan: all chunk state-sums psS_cc = kplc^T v_aug run first
    and are prefix-added into per-chunk S snapshots, making every chunk's
    output matmuls independent (no serial chunk chain).
  * diag_k folded into the kp matmul as an accumulating (-0.5)-matmul on
    sq = kT^2; sq computed per-L-tile right after the k projection.
  * A-blocks skip the always-zero (key-hi x query-lo) quarter: psA is
    [128, 384] = [keys-lo x 256q | keys-hi x 128q-hi].
  * attnT packed 2 heads per [128, L] tile -> wo runs 2x128-contract.
  * den: per-chunk reciprocal + PE ones-broadcast into the nd tile's spare
    bank quadrant; divide fused into the PSUM->SBUF copy of num.
  * Emission schedule: gen(h+1) overlaps scan(h); the last two heads' scans
    are chunk-interleaved with the wo projection folded in, so the tail
    keeps PE/Act/DVE all fed.
"""
import numpy as np

B, L, DIM, H, DK, M = 4, 2048, 512, 8, 64, 256
HPC = 4            # heads per core
CW = 256           # scan chunk width (queries per chunk)
NCC = L // CW      # 8
NC2 = L // 128     # 16
LT = 512
NLT = L // LT

# constant-blob column offsets (bf16 columns)
_OFF_WQ = 0
_OFF_WK = 1024
_OFF_BQK = 2048
_OFF_PROJT = 2052
_OFF_BD = 2308
_OFF_WV = 2316
_OFF_WVB = 3340
_OFF_MSK = 3596
_OFF_WO2 = 3980
_CB = 5004

_COMPILED = None
_DEBUG_ATT = False


def _build():
    import concourse.bacc as bacc
    import concourse.mybir as mybir
    from concourse.tile import TileContext

    f32 = mybir.dt.float32
    bf16 = mybir.dt.bfloat16
    EXP = mybir.ActivationFunctionType.Exp
    RCP = mybir.ActivationFunctionType.Reciprocal

    nc = bacc.Bacc("TRN2", target_bir_lowering=False, debug=False,
                   enable_asserts=False, num_devices=8)

    def din(name, shape, dt=bf16):
        return nc.dram_tensor(name, shape, dt, kind="ExternalInput").ap()

    cblob = din("cblob", [128, _CB])
    xq = din("xq", [512, L])
    xk = din("xk", [512, L])
    xv = din("xv", [512, L])
    outT = nc.dram_tensor("outT", [512, L], bf16, kind="ExternalOutput").ap()
    dbg_att = None
    if _DEBUG_ATT:
        dbg_att = [nc.dram_tensor(f"dbg_att{i}", [128, L], bf16,
                                  kind="ExternalOutput").ap()
                   for i in range(2)]

    with TileContext(nc) as tc, nc.allow_low_precision(
            reason="bf16 scan pipeline, validated 4.8e-3 rel err vs f32 "
                   "reference (tolerance 2e-2)"):
        with (
            tc.tile_pool(name="const", bufs=1) as cpool,
            tc.tile_pool(name="persist", bufs=1) as ppool,
            tc.tile_pool(name="psP", bufs=2, space="PSUM") as psP,
        ):
            # ---- constants: one blob DMA, then slice ----
            blob = cpool.tile([128, _CB], bf16, tag="cblob")
            nc.sync.dma_start(blob[:, :], cblob)

            def bs(off, w):
                return blob[:, off:off + w]

            c_wq = [bs(_OFF_WQ + i * 256, 256) for i in range(4)]
            c_wk = [bs(_OFF_WK + i * 256, 256) for i in range(4)]
            c_bqk = cpool.tile([128, 4], f32, tag="bqk")
            nc.vector.tensor_copy(c_bqk[:, :], bs(_OFF_BQK, 4))
            c_bq = c_bqk[:, 0:2]
            c_bk = c_bqk[:, 2:4]
            c_projT = bs(_OFF_PROJT, 256)
            c_bd = bs(_OFF_BD, 8)
            c_wv = [bs(_OFF_WV + i * 256, 256) for i in range(4)]
            c_wvb = blob[0:1, _OFF_WVB:_OFF_WVB + 256]
            c_msk = bs(_OFF_MSK, 384)
            c_wo2 = [bs(_OFF_WO2 + j * 512, 512) for j in range(2)]
            c_cneg = cpool.tile([128, 128], bf16, tag="cneg")
            nc.any.memset(c_cneg[:, :], -0.5)
            c_ones = cpool.tile([1, 128], bf16, tag="ones")
            nc.any.memset(c_ones[:, :], 1.0)
            c_zS = cpool.tile([128, 132], bf16, tag="zS")
            nc.any.memset(c_zS[:, :], 0.0)
            # bv broadcast [128, 256]: lets the v bias ride the existing
            # PSUM->SBUF copy instead of 16 per-chunk bias matmuls
            psb = psP.tile([128, LT], f32, name="psb", tag="psP")
            nc.tensor.matmul(psb[:, 0:256], c_ones[0:1, 0:128],
                             c_wvb[:, :], start=True, stop=True)
            c_bvb = cpool.tile([128, 256], bf16, tag="bvb")
            nc.vector.tensor_copy(c_bvb[:, :], psb[:, 0:256])

            # persistent activations
            t_qT = [ppool.tile([128, L], bf16, name=f"qT{i}", tag=f"qT{i}")
                    for i in range(2)]
            t_kT = [ppool.tile([128, L], bf16, name=f"kT{i}", tag=f"kT{i}")
                    for i in range(2)]
            t_sq = [ppool.tile([128, L], bf16, name=f"sq{i}", tag=f"sq{i}")
                    for i in range(2)]
            t_v = ppool.tile([128, NC2 * 264], bf16, tag="vall")
            t_ksc = ppool.tile([128, NC2 * 4], f32, tag="ksc")

            # ---- Phase 1: x loads + projections (x tiles persist so head
            # pools can open before the projections finish) ----
            t_x = {}
            for nm, src in (("q", xq), ("k", xk), ("v", xv)):
                for i in range(4):
                    xt = ppool.tile([128, L], bf16, name=f"x{nm}{i}",
                                    tag=f"x{nm}{i}")
                    nc.sync.dma_start(xt[:, :], src[128 * i:128 * (i + 1), :])
                    t_x[(nm, i)] = xt

            # phase-1 psum allocs alternate between the psP tag and the
            # (idle until the scans) psND tag for 4-deep pipelining
            _alt = [0]

            def ps_alt():
                _alt[0] ^= 1
                if _alt[0]:
                    return psND_p.tile([128, LT], f32, name="psx",
                                       tag="psND")
                return psP.tile([128, LT], f32, name="psx", tag="psP")

            def proj_qk(nm, half):
                wgt, dst, bias = ((c_wq, t_qT, c_bq) if nm == "q"
                                  else (c_wk, t_kT, c_bk))
                for lt in range(NLT):
                    ls = slice(lt * LT, (lt + 1) * LT)
                    ps = ps_alt()
                    for kt in range(4):
                        nc.tensor.matmul(
                            ps[:, :],
                            wgt[kt][:, 128 * half:128 * (half + 1)],
                            t_x[(nm, kt)][:, ls],
                            start=(kt == 0), stop=(kt == 3))
                    nc.vector.tensor_scalar_add(dst[half][:, ls],
                                                ps[:, :],
                                                bias[:, half:half + 1])
                    if nm == "k":
                        # sq = kT^2 per-lt on DVE: overlaps projections
                        nc.vector.tensor_mul(t_sq[half][:, ls],
                                             dst[half][:, ls],
                                             dst[half][:, ls])

            def emit_ksc():
                # ksc[l, 4ch+h] = -0.5 * sum_d kT^2  (per-chunk, per-head)
                for ch in range(NC2):
                    cs = slice(ch * 128, (ch + 1) * 128)
                    ps = ps_alt()
                    for half in range(2):
                        nc.tensor.matmul(ps[:, 0:4], t_sq[half][:, cs],
                                         c_bd[:, 4 * half:4 * (half + 1)],
                                         start=(half == 0), stop=(half == 1))
                    nc.scalar.copy(t_ksc[:, 4 * ch:4 * (ch + 1)],
                                   ps[:, 0:4])

            v_r4 = t_v[:, :].rearrange("p (c h x) -> p c h x", h=4, x=66)

            def v_unit(ch):
                # v projection chunk ([l, 4h x 66] layout, ones columns)
                cs = slice(ch * 128, (ch + 1) * 128)
                ps = ps_alt()
                for kt in range(4):
                    nc.tensor.matmul(ps[:, 0:256], t_x[("v", kt)][:, cs],
                                     c_wv[kt][:, :],
                                     start=(kt == 0), stop=(kt == 3))
                ps_r = ps[:, 0:256].rearrange("p (h x) -> p h x", h=4)
                bvb_r = c_bvb[:, :].rearrange("p (h x) -> p h x", h=4)
                nc.vector.tensor_add(v_r4[:, ch, :, 0:64], ps_r[:, :, :],
                                     bvb_r[:, :, :])

            # ---- head pipeline ----
            hctx = (tc.tile_pool(name="headbuf", bufs=3),
                    tc.tile_pool(name="work", bufs=3),
                    tc.tile_pool(name="attn", bufs=1))
            pctx = (tc.tile_pool(name="psA", bufs=2, space="PSUM"),
                    tc.tile_pool(name="psND", bufs=2, space="PSUM"))
            hpool, wpool, apool = [c.__enter__() for c in hctx]
            psA_p, psND_p = [c.__enter__() for c in pctx]
            octx = tc.tile_pool(name="outp", bufs=8)
            opool = octx.__enter__()
            # psS closes after the last prep so its 2 banks can be reused
            # as a second nd pool in the tail (deepens the den pipelines)
            psS_c = tc.tile_pool(name="psS", bufs=2, space="PSUM")
            psS_p = psS_c.__enter__()

            t_att = [apool.tile([128, L], bf16, name=f"att{i}", tag=f"att{i}")
                     for i in range(2)]

            def emit_wo(lt):
                # output projection for one L-tile; psum via the psND tag
                # (the tail scans underload PE/Act, so this fills them)
                ls = slice(lt * LT, (lt + 1) * LT)
                for osub in range(4):
                    os_ = slice(128 * osub, 128 * (osub + 1))
                    # psP banks are idle during the tail (no gen running)
                    ps = psP.tile([128, LT], f32, tag="psP")
                    nc.tensor.matmul(ps[:, :], c_wo2[0][:, os_],
                                     t_att[0][:, ls], start=True, stop=False)
                    nc.tensor.matmul(ps[:, :], c_wo2[1][:, os_],
                                     t_att[1][:, ls], start=False, stop=True)
                    t_o = opool.tile([128, LT], bf16, name="t_o", tag="outT")
                    nc.scalar.copy(t_o[:, :], ps[:, :])
                    nc.sync.dma_start(outT[os_, ls], t_o[:, :])

            heads = {}

            def gen_units(h, alt=False):
                """Allocate head-h tiles; return emission thunk groups
                (qp/kp Exp units, kplc units, prep chunks). alt=True uses
                the alternating psum allocator (only safe before scans)."""
                hh = h // 2
                hr = slice(64 * (h % 2), 64 * (h % 2) + 64)
                pr = hr
                t_qp = [hpool.tile([128, L], bf16, name=f"qp{i}",
                                   tag=f"qp{i}") for i in range(2)]
                t_kp = [hpool.tile([128, L], bf16, name=f"kp{i}",
                                   tag=f"kp{i}") for i in range(2)]
                # kplc is consumed by this head's prep units (end of gen),
                # so 2 bufs suffice even with 3 heads in flight
                t_kplc = hpool.tile([128, NC2 * 256], bf16, tag="kplc",
                                    bufs=2)
                t_S = hpool.tile([128, (NCC - 1) * 132], bf16, tag="S")
                heads[h] = (t_qp, t_kp, t_kplc, t_S)

                def psget():
                    if alt:
                        return ps_alt()
                    return psP.tile([128, LT], f32, name="psx", tag="psP")

                def qp_unit(half, lt):
                    mh = slice(128 * half, 128 * (half + 1))
                    ls = slice(lt * LT, (lt + 1) * LT)
                    ps = psget()
                    nc.tensor.matmul(ps[:, :], c_projT[pr, mh],
                                     t_qT[hh][hr, ls], start=True, stop=True)
                    nc.scalar.activation(t_qp[half][:, ls], ps[:, :], EXP)

                def kp_unit(half, lt):
                    mh = slice(128 * half, 128 * (half + 1))
                    ls = slice(lt * LT, (lt + 1) * LT)
                    ps2 = psget()
                    nc.tensor.matmul(ps2[:, :], c_projT[pr, mh],
                                     t_kT[hh][hr, ls], start=True, stop=False)
                    nc.tensor.matmul(ps2[:, :], c_cneg[pr, :],
                                     t_sq[hh][hr, ls], start=False, stop=True)
                    nc.scalar.activation(t_kp[half][:, ls], ps2[:, :], EXP)

                def kplc_unit(ch):
                    cs = slice(ch * 128, (ch + 1) * 128)
                    ps = psget()
                    nc.tensor.matmul(ps[:, 0:256], t_kT[hh][hr, cs],
                                     c_projT[pr, :], start=True, stop=True)
                    nc.scalar.activation(
                        t_kplc[:, 256 * ch:256 * (ch + 1)], ps[:, 0:256], EXP,
                        bias=t_ksc[:, 4 * ch + h:4 * ch + h + 1])

                qps = [lambda half=half, lt=lt: qp_unit(half, lt)
                       for half in range(2) for lt in range(NLT)]
                kps = [lambda half=half, lt=lt: kp_unit(half, lt)
                       for half in range(2) for lt in range(NLT)]
                kplcs = [lambda ch=ch: kplc_unit(ch) for ch in range(NC2)]
                pst = {}
                preps = [lambda cc=cc: prep_chunk(h, pst, cc)
                         for cc in range(NCC)]
                return qps, kps, kplcs, preps

            def flat_units(h, alt=False):
                # interleaved order: exp units first, state-prep at the end
                qps, kps, kplcs, preps = gen_units(h, alt=alt)
                units = []
                for i in range(8):
                    units.append(qps[i])
                    units.append(kps[i])
                    units.append(kplcs[2 * i])
                    units.append(kplcs[2 * i + 1])
                units.extend(preps)
                return units

            def gen(h):
                for u in flat_units(h):
                    u()

            def va(h, c128):
                o = c128 * 264 + h * 66
                return t_v[:, o:o + 66]

            def prep_chunk(h, store, cc):
                # chunk state-sum + exclusive-prefix snapshot step
                # (the last chunk's own state sum is never consumed)
                t_qp, t_kp, t_kplc, t_S = heads[h]
                if cc < NCC - 1:
                    c0, c1 = 2 * cc, 2 * cc + 1
                    psS = psS_p.tile([128, 132], f32, tag="psS")
                    for mh in range(2):
                        r = slice(66 * mh, 66 * mh + 66)
                        nc.tensor.matmul(
                            psS[:, r],
                            t_kplc[:, c0 * 256 + 128 * mh:
                                   c0 * 256 + 128 * mh + 128],
                            va(h, c0), start=(mh == 0), stop=False)
                        nc.tensor.matmul(
                            psS[:, r],
                            t_kplc[:, c1 * 256 + 128 * mh:
                                   c1 * 256 + 128 * mh + 128],
                            va(h, c1), start=False, stop=(mh == 1))
                    store[(h, cc, 'psS')] = psS
                if cc == 0:
                    return
                dst = t_S[:, (cc - 1) * 132:cc * 132]
                prev = store.pop((h, cc - 1, 'psS'))
                if cc == 1:
                    nc.vector.tensor_copy(dst, prev[:, :])
                else:
                    nc.vector.tensor_add(
                        dst, t_S[:, (cc - 2) * 132:(cc - 1) * 132],
                        prev[:, :])

            def scan_prep(h, store=None):
                if store is None:
                    store = {}
                for cc in range(NCC):
                    prep_chunk(h, store, cc)

            def emit_psA(h, store, cc):
                t_qp, t_kp, _, _ = heads[h]
                qs = slice(cc * CW, (cc + 1) * CW)
                qhi = slice(cc * CW + 128, (cc + 1) * CW)
                klo = slice(cc * CW, cc * CW + 128)
                khi = slice(cc * CW + 128, (cc + 1) * CW)
                # single start..stop bracket per bank: start lazily zeroes
                # the whole 2KB region, so interleaved brackets clobber
                # sibling regions. keys-hi only sees q-hi (keys-hi x q-lo
                # is identically zero under the causal mask).
                psA = psA_p.tile([128, 384], f32, tag="psA")
                nc.tensor.matmul(psA[:, 0:256], t_kp[0][:, klo],
                                 t_qp[0][:, qs], start=True, stop=False)
                nc.tensor.matmul(psA[:, 0:256], t_kp[1][:, klo],
                                 t_qp[1][:, qs], start=False, stop=False)
                nc.tensor.matmul(psA[:, 256:384], t_kp[0][:, khi],
                                 t_qp[0][:, qhi], start=False, stop=False)
                nc.tensor.matmul(psA[:, 256:384], t_kp[1][:, khi],
                                 t_qp[1][:, qhi], start=False, stop=True)
                store[(h, cc, 'psA')] = psA

            def emit_mask(h, store, cc, hold=False):
                # hold=True: buffered atm for a precomputed head (all 8
                # chunks live until the tail consumes them)
                if hold:
                    atm = wpool.tile([128, 384], bf16, name="atm2",
                                     tag="atm2", bufs=8)
                else:
                    atm = wpool.tile([128, 384], bf16, name="atm",
                                     tag="atm", bufs=7)
                psA = store.pop((h, cc, 'psA'))
                nc.vector.tensor_mul(atm[:, :], psA[:, :], c_msk[:, :])
                store[(h, cc, 'atm')] = atm

            def emit_nd(h, store, cc, rcpb_act=False, ndpool=None,
                    rcp_act=False):
                t_qp, t_kp, t_kplc, t_S = heads[h]
                att = t_att[h // 2]
                arow = slice(64 * (h % 2), 64 * (h % 2) + 64)
                qs = slice(cc * CW, (cc + 1) * CW)
                c0, c1 = 2 * cc, 2 * cc + 1
                atm = store.pop((h, cc, 'atm'))
                # full-bank tile: nd in [0:66, 0:256], den-reciprocal
                # broadcast parked in the spare quadrant [64:128, 256:512]
                if ndpool is None:
                    nd = psND_p.tile([128, 512], f32, name="nd", tag="psND")
                else:
                    nd = ndpool.tile([128, 512], f32, name="nd", tag="psX")
                if cc > 0:
                    S_src = t_S[:, (cc - 1) * 132:cc * 132]
                    nc.tensor.matmul(nd[0:66, 0:256], S_src[:, 0:66],
                                     t_qp[0][:, qs], start=True, stop=False)
                    nc.tensor.matmul(nd[0:66, 0:256], S_src[:, 66:132],
                                     t_qp[1][:, qs], start=False, stop=False)
                    nc.tensor.matmul(nd[0:66, 128:256], va(h, c1),
                                     atm[:, 256:384], start=False, stop=False)
                    nc.tensor.matmul(nd[0:66, 0:256], va(h, c0),
                                     atm[:, 0:256], start=False, stop=True)
                else:
                    # chunk 0 has no prior state: va0 opens the bracket
                    # (start lazily zeroes the whole bank), va1 closes it
                    nc.tensor.matmul(nd[0:66, 0:256], va(h, c0),
                                     atm[:, 0:256], start=True, stop=False)
                    nc.tensor.matmul(nd[0:66, 128:256], va(h, c1),
                                     atm[:, 256:384], start=False, stop=True)
                t_rcp = wpool.tile([1, 256], bf16, tag="rcp", bufs=6)
                nc.vector.reciprocal(t_rcp[:, :], nd[64:65, 0:256])
                nc.tensor.matmul(nd[64:128, 256:512], c_ones[0:1, 0:64],
                                 t_rcp[:, :], start=True, stop=True)
                rcpB = wpool.tile([64, 256], bf16, tag="rcpB", bufs=6)
                if rcpb_act:
                    nc.scalar.copy(rcpB[:, :], nd[64:128, 256:512])
                else:
                    nc.vector.tensor_copy(rcpB[:, :], nd[64:128, 256:512])
                nc.vector.tensor_mul(att[arow, qs], nd[0:64, 0:256],
                                     rcpB[:, :])

            def scan(h, units=None, upc=5):
                # single-head scan, psA/mask software-pipelined one chunk
                # ahead so PE never queues behind a mask wait. `units` are
                # gen thunks of a later head, interleaved 5 per chunk so
                # PE/Act stay fed through the scan's dependency stalls.
                # `pre`/`pre_h`: also precompute head pre_h's atm tiles
                # into held buffers (moves the tail's mask DVE work into
                # this Act-bound window, which has DVE slack).
                store = {}
                emit_psA(h, store, 0)
                emit_psA(h, store, 1)
                emit_mask(h, store, 0)
                for cc in range(NCC):
                    if cc + 2 < NCC:
                        emit_psA(h, store, cc + 2)
                    if cc + 1 < NCC:
                        emit_mask(h, store, cc + 1)
                    emit_nd(h, store, cc)
                    if units:
                        for u in units[cc * upc:(cc + 1) * upc]:
                            u()

            def scan_pair(ha, hb, ndpool_b):
                # chunk-interleaved scans of two heads: two independent
                # dependency chains keep all engines fed through the tail;
                # psA/mask run one chunk ahead; each head gets its own
                # 2-buf nd pool so the den chains pipeline 2-deep; wo is
                # folded in as attnT L-tiles complete
                store = {}
                emit_psA(ha, store, 0)
                emit_mask(ha, store, 0)
                emit_psA(hb, store, 0)
                emit_mask(hb, store, 0)
                for cc in range(NCC):
                    if cc + 1 < NCC:
                        emit_psA(ha, store, cc + 1)
                        emit_mask(ha, store, cc + 1)
                        emit_psA(hb, store, cc + 1)
                        emit_mask(hb, store, cc + 1)
                    emit_nd(ha, store, cc, rcpb_act=True)
                    emit_nd(hb, store, cc, rcpb_act=True, ndpool=ndpool_b)
                    if cc % 2 == 1:
                        emit_wo(cc // 2)

            # ---- emission schedule ----
            # gen(0) units start as soon as their half-0 inputs exist so
            # Act ramps early; v_proj interleaves with gen(0)'s kplc units;
            # gen(h) for later heads interleaves into scan(h-2)'s loop.
            nc.any.memset(v_r4[:, :, :, 64:66], 1.0)
            g0qps, g0kps, g0kplcs, g0preps = gen_units(0, alt=True)
            proj_qk("q", 0)
            for u in g0qps:
                u()
            proj_qk("k", 0)
            for u in g0kps:
                u()
            proj_qk("q", 1)
            proj_qk("k", 1)
            emit_ksc()
            for u in g0kplcs:
                u()
            g1units = flat_units(1, alt=True)
            for ch in range(NC2):
                v_unit(ch)
                g1units[2 * ch]()
                g1units[2 * ch + 1]()
            for cc in range(NCC):
                g0preps[cc]()
                g1units[32 + cc]()
            scan(0, units=flat_units(2))
            scan(1, units=flat_units(3))
            # all preps are done: recycle psS's banks as head-b's nd pool
            psS_c.__exit__(None, None, None)
            psX_c = tc.tile_pool(name="psX", bufs=2, space="PSUM")
            psX_p = psX_c.__enter__()
            scan_pair(2, 3, psX_p)
            if _DEBUG_ATT:
                for i in range(2):
                    nc.sync.dma_start(dbg_att[i], t_att[i][:, :])
            psX_c.__exit__(None, None, None)
            octx.__exit__(None, None, None)
            for c in reversed(pctx):
                c.__exit__(None, None, None)
            for c in reversed(hctx):
                c.__exit__(None, None, None)

    nc.compile()
    return nc


def _prep_inputs(query, key, value, Wq, bq, Wk, bk, Wv, bv, Wo, bo, proj):
    from ml_dtypes import bfloat16
    s = float(DK) ** -0.25

    def bf(x):
        return np.ascontiguousarray(x).astype(bfloat16)

    tri = (np.arange(128)[:, None] <= np.arange(128)[None, :]).astype(
        np.float32)
    on = np.ones((128, 128), np.float32)
    msk = np.concatenate([tri, on, tri], axis=1)
    bd = np.zeros((128, 8), np.float32)
    for half in range(2):
        for r in range(128):
            bd[r, 4 * half + (2 * half + r // 64)] = -0.5
    pT = np.ascontiguousarray(proj.T)
    projT2 = np.concatenate([pT, pT])
    in_maps = []
    for b in range(B):
        for hg in range(2):
            sl = slice(hg * 256, (hg + 1) * 256)
            def hpack(mat):
                # [k*128, w] -> [128, k*w]: 128-row tiles side by side
                k = mat.shape[0] // 128
                return np.concatenate([mat[128 * i:128 * (i + 1)]
                                       for i in range(k)], axis=1)

            blob = np.zeros((128, _CB), np.float32)
            blob[:, _OFF_WQ:_OFF_WQ + 1024] = hpack(Wq[sl].T * s)
            blob[:, _OFF_WK:_OFF_WK + 1024] = hpack(Wk[sl].T * s)
            blob[:, _OFF_BQK + 0] = bq[sl][:128] * s
            blob[:, _OFF_BQK + 1] = bq[sl][128:] * s
            blob[:, _OFF_BQK + 2] = bk[sl][:128] * s
            blob[:, _OFF_BQK + 3] = bk[sl][128:] * s
            blob[:, _OFF_PROJT:_OFF_PROJT + 256] = projT2
            blob[:, _OFF_BD:_OFF_BD + 8] = bd
            blob[:, _OFF_WV:_OFF_WV + 1024] = hpack(Wv[sl].T)
            blob[0, _OFF_WVB:_OFF_WVB + 256] = bv[sl]
            blob[:, _OFF_MSK:_OFF_MSK + 384] = msk
            blob[:, _OFF_WO2:_OFF_WO2 + 1024] = hpack(Wo[:, sl].T)
            m = {"cblob": bf(blob),
                 "xq": bf(query[b].T),
                 "xk": bf(key[b].T),
                 "xv": bf(value[b].T)}
            in_maps.append(m)
    return in_maps


def kernel(query, key, value, Wq, bq, Wk, bk, Wv, bv, Wo, bo, proj,
           _trace=False):
    global _COMPILED
    from concourse import bass_utils
    args = [np.asarray(a, np.float32) for a in
            (query, key, value, Wq, bq, Wk, bk, Wv, bv, Wo, bo, proj)]
    if _COMPILED is None:
        _COMPILED = _build()
    in_maps = _prep_inputs(*args)
    res = bass_utils.run_bass_kernel_spmd(
        _COMPILED, in_maps, core_ids=list(range(8)), trace=_trace)
    out = np.empty((B, L, DIM), np.float32)
    bo_ = args[10]
    for b in range(B):
        out[b] = (res.results[2 * b]["outT"].astype(np.float32).T
                  + res.results[2 * b + 1]["outT"].astype(np.float32).T + bo_)
    if _trace:
        kernel._last = res
    return out


# revision 119
# speedup vs baseline: 1.0056x; 1.0056x over previous
"""FAVOR+ (Performer) multi-head causal attention — Trainium2 Bass kernel v2.

Sharding: 8 cores = 4 batches x 2 head-groups (4 heads each).

Math note: the softmax-kernel stabilizers and +eps only rescale qp/kp per
(l,h) [or globally] and cancel in num/den (O(3e-4) perturbation). We compute
raw exp(dd) for Q and exp(dd - diag_k) for K; no collectives needed.

v2 design:
  * All scan-side math in bf16 (PSUM accumulates f32). Host ships x and
    weights pre-converted to bf16: no on-chip rounding copies, half the DMA.
    Validated numerically: 4.8e-3 rel err vs f32 reference (tol 2e-2).
  * All small constants packed into ONE blob DMA (SP SEQ issues DMAs at
    565ns each, so DMA count gates the pipeline start).
  * Hierarchical scan: all chunk state-sums psS_cc = kplc^T v_aug run first
    and are prefix-added into per-chunk S snapshots, making every chunk's
    output matmuls independent (no serial chunk chain).
  * diag_k folded into the kp matmul as an accumulating (-0.5)-matmul on
    sq = kT^2; sq computed per-L-tile right after the k projection.
  * A-blocks skip the always-zero (key-hi x query-lo) quarter: psA is
    [128, 384] = [keys-lo x 256q | keys-hi x 128q-hi].
  * attnT packed 2 heads per [128, L] tile -> wo runs 2x128-contract.
  * den: per-chunk reciprocal + PE ones-broadcast into the nd tile's spare
    bank quadrant; divide fused into the PSUM->SBUF copy of num.
  * Emission schedule: gen(h+1) overlaps scan(h); the last two heads' scans
    are chunk-interleaved with the wo projection folded in, so the tail
    keeps PE/Act/DVE all fed.
"""
import numpy as np

B, L, DIM, H, DK, M = 4, 2048, 512, 8, 64, 256
HPC = 4            # heads per core
CW = 256           # scan chunk width (queries per chunk)
NCC = L // CW      # 8
NC2 = L // 128     # 16
LT = 512
NLT = L // LT

# constant-blob column offsets (bf16 columns)
_OFF_WQ = 0
_OFF_WK = 1024
_OFF_BQK = 2048
_OFF_PROJT = 2052
_OFF_BD = 2308
_OFF_WV = 2316
_OFF_WVB = 3340
_OFF_MSK = 3596
_OFF_WO2 = 3980
_CB = 5004

_COMPILED = None
_DEBUG_ATT = False


def _build():
    import concourse.bacc as bacc
    import concourse.mybir as mybir
    from concourse.tile import TileContext

    f32 = mybir.dt.float32
    bf16 = mybir.dt.bfloat16
    EXP = mybir.ActivationFunctionType.Exp
    RCP = mybir.ActivationFunctionType.Reciprocal

    nc = bacc.Bacc("TRN2", target_bir_lowering=False, debug=False,
                   enable_asserts=False, num_devices=8)

    def din(name, shape, dt=bf16):
        return nc.dram_tensor(name, shape, dt, kind="ExternalInput").ap()

    cblob = din("cblob", [128, _CB])
    xq = din("xq", [512, L])
    xk = din("xk", [512, L])
    xv = din("xv", [512, L])
    outT = nc.dram_tensor("outT", [512, L], bf16, kind="ExternalOutput").ap()
    dbg_att = None
    if _DEBUG_ATT:
        dbg_att = [nc.dram_tensor(f"dbg_att{i}", [128, L], bf16,
                                  kind="ExternalOutput").ap()
                   for i in range(2)]

    with TileContext(nc) as tc, nc.allow_low_precision(
            reason="bf16 scan pipeline, validated 4.8e-3 rel err vs f32 "
                   "reference (tolerance 2e-2)"):
        with (
            tc.tile_pool(name="const", bufs=1) as cpool,
            tc.tile_pool(name="persist", bufs=1) as ppool,
            tc.tile_pool(name="psP", bufs=2, space="PSUM") as psP,
        ):
            # ---- constants: one blob DMA, then slice ----
            blob = cpool.tile([128, _CB], bf16, tag="cblob")
            # split transfer: only cols [0, _OFF_PROJT) (wq/wk/biases) gate
            # the first matmuls; the rest is DMA'd after the xq tiles
            nc.sync.dma_start(blob[:, 0:_OFF_PROJT], cblob[:, 0:_OFF_PROJT])

            def bs(off, w):
                return blob[:, off:off + w]

            c_wq = [bs(_OFF_WQ + i * 256, 256) for i in range(4)]
            c_wk = [bs(_OFF_WK + i * 256, 256) for i in range(4)]
            c_bqk = cpool.tile([128, 4], f32, tag="bqk")
            nc.vector.tensor_copy(c_bqk[:, :], bs(_OFF_BQK, 4))
            c_bq = c_bqk[:, 0:2]
            c_bk = c_bqk[:, 2:4]
            c_projT = bs(_OFF_PROJT, 256)
            c_bd = bs(_OFF_BD, 8)
            c_wv = [bs(_OFF_WV + i * 256, 256) for i in range(4)]
            c_wvb = blob[0:1, _OFF_WVB:_OFF_WVB + 256]
            c_msk = bs(_OFF_MSK, 384)
            c_wo2 = [bs(_OFF_WO2 + j * 512, 512) for j in range(2)]
            c_cneg = cpool.tile([128, 128], bf16, tag="cneg")
            nc.any.memset(c_cneg[:, :], -0.5)
            c_ones = cpool.tile([1, 128], bf16, tag="ones")
            nc.any.memset(c_ones[:, :], 1.0)
            c_zS = cpool.tile([128, 132], bf16, tag="zS")
            nc.any.memset(c_zS[:, :], 0.0)

            # persistent activations
            t_qT = [ppool.tile([128, L], bf16, name=f"qT{i}", tag=f"qT{i}")
                    for i in range(2)]
            t_kT = [ppool.tile([128, L], bf16, name=f"kT{i}", tag=f"kT{i}")
                    for i in range(2)]
            t_sq = [ppool.tile([128, L], bf16, name=f"sq{i}", tag=f"sq{i}")
                    for i in range(2)]
            t_v = ppool.tile([128, NC2 * 264], bf16, tag="vall")
            t_ksc = ppool.tile([128, NC2 * 4], f32, tag="ksc")

            # ---- Phase 1: x loads + projections (x tiles persist so head
            # pools can open before the projections finish) ----
            t_x = {}
            for nm, src in (("q", xq), ("k", xk), ("v", xv)):
                for i in range(4):
                    xt = ppool.tile([128, L], bf16, name=f"x{nm}{i}",
                                    tag=f"x{nm}{i}")
                    nc.sync.dma_start(xt[:, :], src[128 * i:128 * (i + 1), :])
                    t_x[(nm, i)] = xt
                if nm == "q" and i == 3:
                    nc.sync.dma_start(blob[:, _OFF_PROJT:_CB],
                                      cblob[:, _OFF_PROJT:_CB])

            # bv broadcast [128, 256]: lets the v bias ride the existing
            # PSUM->SBUF copy instead of 16 per-chunk bias matmuls.
            # (emitted AFTER the second blob DMA: it reads cols >= _OFF_WVB)
            psb = psP.tile([128, LT], f32, name="psb", tag="psP")
            nc.tensor.matmul(psb[:, 0:256], c_ones[0:1, 0:128],
                             c_wvb[:, :], start=True, stop=True)
            c_bvb = cpool.tile([128, 256], bf16, tag="bvb")
            nc.vector.tensor_copy(c_bvb[:, :], psb[:, 0:256])

            # phase-1 psum allocs alternate between the psP tag and the
            # (idle until the scans) psND tag for 4-deep pipelining
            _alt = [0]

            def ps_alt():
                _alt[0] ^= 1
                if _alt[0]:
                    return psND_p.tile([128, LT], f32, name="psx",
                                       tag="psND")
                return psP.tile([128, LT], f32, name="psx", tag="psP")

            def proj_qk(nm, half):
                wgt, dst, bias = ((c_wq, t_qT, c_bq) if nm == "q"
                                  else (c_wk, t_kT, c_bk))
                for lt in range(NLT):
                    ls = slice(lt * LT, (lt + 1) * LT)
                    ps = ps_alt()
                    for kt in range(4):
                        nc.tensor.matmul(
                            ps[:, :],
                            wgt[kt][:, 128 * half:128 * (half + 1)],
                            t_x[(nm, kt)][:, ls],
                            start=(kt == 0), stop=(kt == 3))
                    nc.vector.tensor_scalar_add(dst[half][:, ls],
                                                ps[:, :],
                                                bias[:, half:half + 1])
                    if nm == "k":
                        # sq = kT^2 per-lt on DVE: overlaps projections
                        nc.vector.tensor_mul(t_sq[half][:, ls],
                                             dst[half][:, ls],
                                             dst[half][:, ls])

            def emit_ksc():
                # ksc[l, 4ch+h] = -0.5 * sum_d kT^2  (per-chunk, per-head)
                for ch in range(NC2):
                    cs = slice(ch * 128, (ch + 1) * 128)
                    ps = ps_alt()
                    for half in range(2):
                        nc.tensor.matmul(ps[:, 0:4], t_sq[half][:, cs],
                                         c_bd[:, 4 * half:4 * (half + 1)],
                                         start=(half == 0), stop=(half == 1))
                    nc.scalar.copy(t_ksc[:, 4 * ch:4 * (ch + 1)],
                                   ps[:, 0:4])

            v_r4 = t_v[:, :].rearrange("p (c h x) -> p c h x", h=4, x=66)

            def v_unit(ch):
                # v projection chunk ([l, 4h x 66] layout, ones columns)
                cs = slice(ch * 128, (ch + 1) * 128)
                ps = ps_alt()
                for kt in range(4):
                    nc.tensor.matmul(ps[:, 0:256], t_x[("v", kt)][:, cs],
                                     c_wv[kt][:, :],
                                     start=(kt == 0), stop=(kt == 3))
                ps_r = ps[:, 0:256].rearrange("p (h x) -> p h x", h=4)
                bvb_r = c_bvb[:, :].rearrange("p (h x) -> p h x", h=4)
                nc.vector.tensor_add(v_r4[:, ch, :, 0:64], ps_r[:, :, :],
                                     bvb_r[:, :, :])

            # ---- head pipeline ----
            hctx = (tc.tile_pool(name="headbuf", bufs=3),
                    tc.tile_pool(name="work", bufs=3),
                    tc.tile_pool(name="attn", bufs=1))
            pctx = (tc.tile_pool(name="psA", bufs=2, space="PSUM"),
                    tc.tile_pool(name="psND", bufs=2, space="PSUM"))
            hpool, wpool, apool = [c.__enter__() for c in hctx]
            psA_p, psND_p = [c.__enter__() for c in pctx]
            octx = tc.tile_pool(name="outp", bufs=8)
            opool = octx.__enter__()
            # psS closes after the last prep so its 2 banks can be reused
            # as a second nd pool in the tail (deepens the den pipelines)
            psS_c = tc.tile_pool(name="psS", bufs=2, space="PSUM")
            psS_p = psS_c.__enter__()

            t_att = [apool.tile([128, L], bf16, name=f"att{i}", tag=f"att{i}")
                     for i in range(2)]

            def emit_wo(lt):
                # output projection for one L-tile; psum via the psND tag
                # (the tail scans underload PE/Act, so this fills them)
                ls = slice(lt * LT, (lt + 1) * LT)
                for osub in range(4):
                    os_ = slice(128 * osub, 128 * (osub + 1))
                    # psP banks are idle during the tail (no gen running)
                    ps = psP.tile([128, LT], f32, tag="psP")
                    nc.tensor.matmul(ps[:, :], c_wo2[0][:, os_],
                                     t_att[0][:, ls], start=True, stop=False)
                    nc.tensor.matmul(ps[:, :], c_wo2[1][:, os_],
                                     t_att[1][:, ls], start=False, stop=True)
                    t_o = opool.tile([128, LT], bf16, name="t_o", tag="outT")
                    nc.scalar.copy(t_o[:, :], ps[:, :])
                    nc.sync.dma_start(outT[os_, ls], t_o[:, :])

            heads = {}

            def gen_units(h, alt=False):
                """Allocate head-h tiles; return emission thunk groups
                (qp/kp Exp units, kplc units, prep chunks). alt=True uses
                the alternating psum allocator (only safe before scans)."""
                hh = h // 2
                hr = slice(64 * (h % 2), 64 * (h % 2) + 64)
                pr = hr
                t_qp = [hpool.tile([128, L], bf16, name=f"qp{i}",
                                   tag=f"qp{i}") for i in range(2)]
                t_kp = [hpool.tile([128, L], bf16, name=f"kp{i}",
                                   tag=f"kp{i}") for i in range(2)]
                # kplc is consumed by this head's prep units (end of gen),
                # so 2 bufs suffice even with 3 heads in flight
                t_kplc = hpool.tile([128, NC2 * 256], bf16, tag="kplc",
                                    bufs=2)
                t_S = hpool.tile([128, (NCC - 1) * 132], bf16, tag="S")
                heads[h] = (t_qp, t_kp, t_kplc, t_S)

                def psget():
                    if alt:
                        return ps_alt()
                    return psP.tile([128, LT], f32, name="psx", tag="psP")

                def qp_unit(half, lt):
                    mh = slice(128 * half, 128 * (half + 1))
                    ls = slice(lt * LT, (lt + 1) * LT)
                    ps = psget()
                    nc.tensor.matmul(ps[:, :], c_projT[pr, mh],
                                     t_qT[hh][hr, ls], start=True, stop=True)
                    nc.scalar.activation(t_qp[half][:, ls], ps[:, :], EXP)

                def kp_unit(half, lt):
                    mh = slice(128 * half, 128 * (half + 1))
                    ls = slice(lt * LT, (lt + 1) * LT)
                    ps2 = psget()
                    nc.tensor.matmul(ps2[:, :], c_projT[pr, mh],
                                     t_kT[hh][hr, ls], start=True, stop=False)
                    nc.tensor.matmul(ps2[:, :], c_cneg[pr, :],
                                     t_sq[hh][hr, ls], start=False, stop=True)
                    nc.scalar.activation(t_kp[half][:, ls], ps2[:, :], EXP)

                def kplc_unit(ch):
                    cs = slice(ch * 128, (ch + 1) * 128)
                    ps = psget()
                    nc.tensor.matmul(ps[:, 0:256], t_kT[hh][hr, cs],
                                     c_projT[pr, :], start=True, stop=True)
                    nc.scalar.activation(
                        t_kplc[:, 256 * ch:256 * (ch + 1)], ps[:, 0:256], EXP,
                        bias=t_ksc[:, 4 * ch + h:4 * ch + h + 1])

                qps = [lambda half=half, lt=lt: qp_unit(half, lt)
                       for half in range(2) for lt in range(NLT)]
                kps = [lambda half=half, lt=lt: kp_unit(half, lt)
                       for half in range(2) for lt in range(NLT)]
                kplcs = [lambda ch=ch: kplc_unit(ch) for ch in range(NC2)]
                pst = {}
                preps = [lambda cc=cc: prep_chunk(h, pst, cc)
                         for cc in range(NCC)]
                return qps, kps, kplcs, preps

            def flat_units(h, alt=False):
                # interleaved order: exp units first, state-prep at the end
                qps, kps, kplcs, preps = gen_units(h, alt=alt)
                units = []
                for i in range(8):
                    units.append(qps[i])
                    units.append(kps[i])
                    units.append(kplcs[2 * i])
                    units.append(kplcs[2 * i + 1])
                units.extend(preps)
                return units

            def gen(h):
                for u in flat_units(h):
                    u()

            def va(h, c128):
                o = c128 * 264 + h * 66
                return t_v[:, o:o + 66]

            def prep_chunk(h, store, cc):
                # chunk state-sum + exclusive-prefix snapshot step
                # (the last chunk's own state sum is never consumed)
                t_qp, t_kp, t_kplc, t_S = heads[h]
                if cc < NCC - 1:
                    c0, c1 = 2 * cc, 2 * cc + 1
                    psS = psS_p.tile([128, 132], f32, tag="psS")
                    for mh in range(2):
                        r = slice(66 * mh, 66 * mh + 66)
                        nc.tensor.matmul(
                            psS[:, r],
                            t_kplc[:, c0 * 256 + 128 * mh:
                                   c0 * 256 + 128 * mh + 128],
                            va(h, c0), start=(mh == 0), stop=False)
                        nc.tensor.matmul(
                            psS[:, r],
                            t_kplc[:, c1 * 256 + 128 * mh:
                                   c1 * 256 + 128 * mh + 128],
                            va(h, c1), start=False, stop=(mh == 1))
                    store[(h, cc, 'psS')] = psS
                if cc == 0:
                    return
                dst = t_S[:, (cc - 1) * 132:cc * 132]
                prev = store.pop((h, cc - 1, 'psS'))
                if cc == 1:
                    nc.vector.tensor_copy(dst, prev[:, :])
                else:
                    nc.vector.tensor_add(
                        dst, t_S[:, (cc - 2) * 132:(cc - 1) * 132],
                        prev[:, :])

            def scan_prep(h, store=None):
                if store is None:
                    store = {}
                for cc in range(NCC):
                    prep_chunk(h, store, cc)

            def emit_psA(h, store, cc):
                t_qp, t_kp, _, _ = heads[h]
                qs = slice(cc * CW, (cc + 1) * CW)
                qhi = slice(cc * CW + 128, (cc + 1) * CW)
                klo = slice(cc * CW, cc * CW + 128)
                khi = slice(cc * CW + 128, (cc + 1) * CW)
                # single start..stop bracket per bank: start lazily zeroes
                # the whole 2KB region, so interleaved brackets clobber
                # sibling regions. keys-hi only sees q-hi (keys-hi x q-lo
                # is identically zero under the causal mask).
                psA = psA_p.tile([128, 384], f32, tag="psA")
                nc.tensor.matmul(psA[:, 0:256], t_kp[0][:, klo],
                                 t_qp[0][:, qs], start=True, stop=False)
                nc.tensor.matmul(psA[:, 0:256], t_kp[1][:, klo],
                                 t_qp[1][:, qs], start=False, stop=False)
                nc.tensor.matmul(psA[:, 256:384], t_kp[0][:, khi],
                                 t_qp[0][:, qhi], start=False, stop=False)
                nc.tensor.matmul(psA[:, 256:384], t_kp[1][:, khi],
                                 t_qp[1][:, qhi], start=False, stop=True)
                store[(h, cc, 'psA')] = psA

            def emit_mask(h, store, cc, hold=False):
                # hold=True: buffered atm for a precomputed head (all 8
                # chunks live until the tail consumes them)
                if hold:
                    atm = wpool.tile([128, 384], bf16, name="atm2",
                                     tag="atm2", bufs=8)
                else:
                    atm = wpool.tile([128, 384], bf16, name="atm",
                                     tag="atm", bufs=7)
                psA = store.pop((h, cc, 'psA'))
                nc.vector.tensor_mul(atm[:, :], psA[:, :], c_msk[:, :])
                store[(h, cc, 'atm')] = atm

            def emit_nd(h, store, cc, rcpb_act=False, ndpool=None,
                    rcp_act=False):
                t_qp, t_kp, t_kplc, t_S = heads[h]
                att = t_att[h // 2]
                arow = slice(64 * (h % 2), 64 * (h % 2) + 64)
                qs = slice(cc * CW, (cc + 1) * CW)
                c0, c1 = 2 * cc, 2 * cc + 1
                atm = store.pop((h, cc, 'atm'))
                # full-bank tile: nd in [0:66, 0:256], den-reciprocal
                # broadcast parked in the spare quadrant [64:128, 256:512]
                if ndpool is None:
                    nd = psND_p.tile([128, 512], f32, name="nd", tag="psND")
                else:
                    nd = ndpool.tile([128, 512], f32, name="nd", tag="psX")
                if cc > 0:
                    S_src = t_S[:, (cc - 1) * 132:cc * 132]
                    nc.tensor.matmul(nd[0:66, 0:256], S_src[:, 0:66],
                                     t_qp[0][:, qs], start=True, stop=False)
                    nc.tensor.matmul(nd[0:66, 0:256], S_src[:, 66:132],
                                     t_qp[1][:, qs], start=False, stop=False)
                    nc.tensor.matmul(nd[0:66, 128:256], va(h, c1),
                                     atm[:, 256:384], start=False, stop=False)
                    nc.tensor.matmul(nd[0:66, 0:256], va(h, c0),
                                     atm[:, 0:256], start=False, stop=True)
                else:
                    # chunk 0 has no prior state: va0 opens the bracket
                    # (start lazily zeroes the whole bank), va1 closes it
                    nc.tensor.matmul(nd[0:66, 0:256], va(h, c0),
                                     atm[:, 0:256], start=True, stop=False)
                    nc.tensor.matmul(nd[0:66, 128:256], va(h, c1),
                                     atm[:, 256:384], start=False, stop=True)
                t_rcp = wpool.tile([1, 256], bf16, tag="rcp", bufs=6)
                nc.vector.reciprocal(t_rcp[:, :], nd[64:65, 0:256])
                nc.tensor.matmul(nd[64:128, 256:512], c_ones[0:1, 0:64],
                                 t_rcp[:, :], start=True, stop=True)
                rcpB = wpool.tile([64, 256], bf16, tag="rcpB", bufs=6)
                if rcpb_act:
                    nc.scalar.copy(rcpB[:, :], nd[64:128, 256:512])
                else:
                    nc.vector.tensor_copy(rcpB[:, :], nd[64:128, 256:512])
                nc.vector.tensor_mul(att[arow, qs], nd[0:64, 0:256],
                                     rcpB[:, :])

            def scan(h, units=None, upc=5):
                # single-head scan, psA/mask software-pipelined one chunk
                # ahead so PE never queues behind a mask wait. `units` are
                # gen thunks of a later head, interleaved 5 per chunk so
                # PE/Act stay fed through the scan's dependency stalls.
                # `pre`/`pre_h`: also precompute head pre_h's atm tiles
                # into held buffers (moves the tail's mask DVE work into
                # this Act-bound window, which has DVE slack).
                store = {}
                emit_psA(h, store, 0)
                emit_psA(h, store, 1)
                emit_mask(h, store, 0)
                for cc in range(NCC):
                    if cc + 2 < NCC:
                        emit_psA(h, store, cc + 2)
                    if cc + 1 < NCC:
                        emit_mask(h, store, cc + 1)
                    emit_nd(h, store, cc)
                    if units:
                        for u in units[cc * upc:(cc + 1) * upc]:
                            u()

            def scan_pair(ha, hb, ndpool_b):
                # chunk-interleaved scans of two heads: two independent
                # dependency chains keep all engines fed through the tail;
                # psA/mask run one chunk ahead; each head gets its own
                # 2-buf nd pool so the den chains pipeline 2-deep; wo is
                # folded in as attnT L-tiles complete
                store = {}
                emit_psA(ha, store, 0)
                emit_mask(ha, store, 0)
                emit_psA(hb, store, 0)
                emit_mask(hb, store, 0)
                for cc in range(NCC):
                    if cc + 1 < NCC:
                        emit_psA(ha, store, cc + 1)
                        emit_mask(ha, store, cc + 1)
                        emit_psA(hb, store, cc + 1)
                        emit_mask(hb, store, cc + 1)
                    emit_nd(ha, store, cc, rcpb_act=True)
                    emit_nd(hb, store, cc, rcpb_act=True, ndpool=ndpool_b)
                    if cc % 2 == 1:
                        emit_wo(cc // 2)

            # ---- emission schedule ----
            # gen(0) units start as soon as their half-0 inputs exist so
            # Act ramps early; v_proj interleaves with gen(0)'s kplc units;
            # gen(h) for later heads interleaves into scan(h-2)'s loop.
            nc.any.memset(v_r4[:, :, :, 64:66], 1.0)
            g0qps, g0kps, g0kplcs, g0preps = gen_units(0, alt=True)
            proj_qk("q", 0)
            for u in g0qps:
                u()
            proj_qk("k", 0)
            for u in g0kps:
                u()
            proj_qk("q", 1)
            proj_qk("k", 1)
            emit_ksc()
            for u in g0kplcs:
                u()
            g1units = flat_units(1, alt=True)
            for ch in range(NC2):
                v_unit(ch)
                g1units[2 * ch]()
                g1units[2 * ch + 1]()
            for cc in range(NCC):
                g0preps[cc]()
                g1units[32 + cc]()
            scan(0, units=flat_units(2))
            scan(1, units=flat_units(3))
            # all preps are done: recycle psS's banks as head-b's nd pool
            psS_c.__exit__(None, None, None)
            psX_c = tc.tile_pool(name="psX", bufs=2, space="PSUM")
            psX_p = psX_c.__enter__()
            scan_pair(2, 3, psX_p)
            if _DEBUG_ATT:
                for i in range(2):
                    nc.sync.dma_start(dbg_att[i], t_att[i][:, :])
            psX_c.__exit__(None, None, None)
            octx.__exit__(None, None, None)
            for c in reversed(pctx):
                c.__exit__(None, None, None)
            for c in reversed(hctx):
                c.__exit__(None, None, None)

    nc.compile()
    return nc


def _prep_inputs(query, key, value, Wq, bq, Wk, bk, Wv, bv, Wo, bo, proj):
    from ml_dtypes import bfloat16
    s = float(DK) ** -0.25

    def bf(x):
        return np.ascontiguousarray(x).astype(bfloat16)

    tri = (np.arange(128)[:, None] <= np.arange(128)[None, :]).astype(
        np.float32)
    on = np.ones((128, 128), np.float32)
    msk = np.concatenate([tri, on, tri], axis=1)
    bd = np.zeros((128, 8), np.float32)
    for half in range(2):
        for r in range(128):
            bd[r, 4 * half + (2 * half + r // 64)] = -0.5
    pT = np.ascontiguousarray(proj.T)
    projT2 = np.concatenate([pT, pT])
    in_maps = []
    for b in range(B):
        for hg in range(2):
            sl = slice(hg * 256, (hg + 1) * 256)
            def hpack(mat):
                # [k*128, w] -> [128, k*w]: 128-row tiles side by side
                k = mat.shape[0] // 128
                return np.concatenate([mat[128 * i:128 * (i + 1)]
                                       for i in range(k)], axis=1)

            blob = np.zeros((128, _CB), np.float32)
            blob[:, _OFF_WQ:_OFF_WQ + 1024] = hpack(Wq[sl].T * s)
            blob[:, _OFF_WK:_OFF_WK + 1024] = hpack(Wk[sl].T * s)
            blob[:, _OFF_BQK + 0] = bq[sl][:128] * s
            blob[:, _OFF_BQK + 1] = bq[sl][128:] * s
            blob[:, _OFF_BQK + 2] = bk[sl][:128] * s
            blob[:, _OFF_BQK + 3] = bk[sl][128:] * s
            blob[:, _OFF_PROJT:_OFF_PROJT + 256] = projT2
            blob[:, _OFF_BD:_OFF_BD + 8] = bd
            blob[:, _OFF_WV:_OFF_WV + 1024] = hpack(Wv[sl].T)
            blob[0, _OFF_WVB:_OFF_WVB + 256] = bv[sl]
            blob[:, _OFF_MSK:_OFF_MSK + 384] = msk
            blob[:, _OFF_WO2:_OFF_WO2 + 1024] = hpack(Wo[:, sl].T)
            m = {"cblob": bf(blob),
                 "xq": bf(query[b].T),
                 "xk": bf(key[b].T),
                 "xv": bf(value[b].T)}
            in_maps.append(m)
    return in_maps


def kernel(query, key, value, Wq, bq, Wk, bk, Wv, bv, Wo, bo, proj,
           _trace=False):
    global _COMPILED
    from concourse import bass_utils
    args = [np.asarray(a, np.float32) for a in
            (query, key, value, Wq, bq, Wk, bk, Wv, bv, Wo, bo, proj)]
    if _COMPILED is None:
        _COMPILED = _build()
    in_maps = _prep_inputs(*args)
    res = bass_utils.run_bass_kernel_spmd(
        _COMPILED, in_maps, core_ids=list(range(8)), trace=_trace)
    out = np.empty((B, L, DIM), np.float32)
    bo_ = args[10]
    for b in range(B):
        out[b] = (res.results[2 * b]["outT"].astype(np.float32).T
                  + res.results[2 * b + 1]["outT"].astype(np.float32).T + bo_)
    if _trace:
        kernel._last = res
    return out
